# revision 37
# baseline (speedup 1.0000x reference)
"""Multi-head attention (nn_Attention_987842478290) on 8 TRN2 NeuronCores.

Sharding: batch (2) x head-group (4 groups of 4 heads) = 8 cores; the host
pre-transposes q/k/v per batch and slices Wq/Wk/Wv columns (and Wo rows)
per head group, so every core runs the identical SPMD program on its shard.

Per core (one batch, 4 heads as 2 "hp" pairs):
  - q/k/v projections on PE (bf16 in, fp32 PSUM); qh/kh kept TRANSPOSED
    [head-cols, tokens] in f32r.
  - scores per (group, m-tile): S = [m128, headA-n512 | headB-n512], two
    K=64 matmuls into a dedicated 2-deep PSUM ring (banks 0-3).
  - exp: ACT (exp, scale=1/8 folded) or DVE fast-exp -- a bf16 bit-trick
    (tensor_scalar mult+add into an int16 view of the pt tile, ~1.8% rms
    error, bias-centered) -- chosen greedily per tile by estimated engine
    completion so the exp stream never paces PE; DVE share capped for
    numerics (~45% measured, rel err 8.4e-3 vs 2e-2 budget).
  - AV pt-stationary: acc[(chunk,hb), c64] accumulated over 16 m-tiles in
    ONE PSUM bank per group (banks 6-7 alternate); softmax row-sums are
    accumulated by 1-row ones-matmuls into a 16-word corner of bank 5
    (start=False + DVE memset: a bank-wide start=True zero-region would
    clobber co-tenants).
  - normalization: DVE reciprocal + per-partition scalar multiplies, then
    a PE transpose matmul (is_transpose, identity moving operand) into a
    re-zeroed slot of the just-drained acc bank, staged to SBUF attT by an
    ACT/DVE copy. NOTE: a PE transpose corrupts any concurrently-OPEN
    accumulation group in its own PSUM bank (walrus/birsim behavior), so
    transposes only ever target banks whose groups are all closed; they
    are queued at phase end and emitted one per step of the next phase.
  - out-projection per 128-token chunk: mid-kernel via bank-4 [P,512]
    halves (PE) staged bf16 by ACT/DVE copies, DMA'd out from alternating
    SP/Pool queues (one queue serializes: each DMA holds its issuing
    engine ~2.3us); the last j-block runs in the epilogue through the
    freed S-ring banks as full-width tiles.

Scheduling: a single-pass emitter with per-engine virtual clocks. Filler
matmul quanta (projection / out-projection single matmuls) are
rate-controlled between the S-tile matmuls under an EDF policy with
per-bank open slots and readiness estimates, so PE stays dense while the
exp stream consumes the S ring; prerequisites are force-emitted (with a
finite horizon so never-emitted producers are not queued early, which
would head-of-line block the PE). Startup interleaves j0 k/q quarter DMA
loads with both prologue projection streams on separate PSUM banks; the
epilogue runs the last group's AV as a dense burst, drains with low
latency PE transposes, and pipelines the final out-proj per chunk.

Host: sums the 8 partial outputs per batch and adds bo + bv @ Wo.
TimelineSim: 175.5us (baseline 207.1us). Rel err vs fp32 ref: 8.4e-3.
"""

import math

import numpy as np
import ml_dtypes

import concourse.bass as bass
import concourse.mybir as mybir
import concourse.tile as tile
from concourse.bass_utils import run_bass_kernel_spmd
from concourse.vector_clock import ScopedClock

F32 = mybir.dt.float32
F32R = mybir.dt.float32r
BF16 = mybir.dt.bfloat16
I16 = mybir.dt.int16
AF = mybir.ActivationFunctionType
ALU = mybir.AluOpType

B, T, E = 2, 2048, 1024
HEADS, HD = 16, 64
NC_ = 8
GROUPS = 4                  # head-groups (4 heads each)
GC = 256                    # cols per core = 4 heads * 64
P = 128
KC = E // P                 # 8 contraction chunks for projections
NJ = T // 512               # 4 n-chunks of 512
SCALE = 1.0 / np.sqrt(HD)   # 1/8
GS = [(j, hp) for j in range(NJ) for hp in range(2)]  # group sequence

# fast-exp (DVE) constants: bf16 bits = trunc(x*scale*128*log2e + bias)
FE_SCALE = float(SCALE * 128.0 / math.log(2.0))
FE_BIAS = 16248.9
MAX_FE_HALF = 160            # numerics cap, in half-tiles
# cumulative fast-exp budget by phase (half-tiles): save quota for tails
FE_CAP = [20, 40, 60, 80, 100, 120, 140, 160]

# cost-model constants for the emitter's virtual clocks (ns)
PE_ROW = 0.4167
ACT_TILE = 1105.0           # one [128,1024] exp incl per-inst overheads
DVE_FE = 1260.0             # fast-exp tensor_scalar on DVE
DMA_BPN = 22.4              # DMA bytes/ns per engine (x16 engines)
SEM = 900.0                 # DMA sem propagation
EPS = 40.0
INF = 1e18
SPLIT_EXP = False


class SplitDrainTileContext(tile.TileContext):
    """TileContext whose final drain never carries >1 sem wait.

    This walrus build rejects >1 sync-wait per instruction; the stock
    epilogue funnels every outstanding wait onto one SP Drain. Emit the
    extra waits on individual SP nops instead.
    """

    def _drain_and_barrier(self, tick_clock, wait_clock):
        drain_inst = self.nc.sync.drain()
        wait_clock.add_sem_waits(
            drain_inst.ins, ScopedClock({None: tick_clock.global_clock})
        )
        si = drain_inst.ins.sync_info
        waits = list(si.on_wait) if si is not None else []
        if len(waits) > 1:
            import bass_rust

            si.on_wait = waits[:1]
            for w in waits[1:]:
                nop = self.nc.sync.nop(nofuse=True)
                nop.ins.sync_info = bass_rust.SyncInfo(on_wait=[w], on_update=[])

        self.nc.all_engine_barrier()
        assert self.sems is not None
        popped = self.nc._tile_sem_poison_stack.pop()
        assert popped is self._sem_poison
        self.nc.clear_and_free_semaphores(list(self.sems.allocated().values()))
        self.nc.all_engine_barrier()


def _split_multi_waits(nc):
    """Move excess sem waits onto preceding same-engine nops.

    This walrus build accepts at most one sync wait per instruction (two
    for EventSemaphore); Tile's scheduler sometimes attaches more (final
    drain, DMA WAR chains). Each engine executes its block instructions
    in list order, so a nop carrying the extra wait immediately before
    the instruction preserves semantics.
    """
    import bass_rust

    for f in nc.m.functions:
        for bb in f.blocks:
            insts = list(bb.instructions)
            out, changed = [], False
            for inst in insts:
                si = inst.sync_info
                waits = list(si.on_wait) if si is not None else []
                cap = 2 if isinstance(inst, mybir.InstEventSemaphore) else 1
                if len(waits) > cap:
                    changed = True
                    for w in waits[: len(waits) - cap]:
                        nop = mybir.InstNoOp(
                            name=f"I-splitw-{nc.next_id()}",
                            ins=[],
                            outs=[],
                        )
                        nop.engine = inst.engine
                        nop.sync_info = bass_rust.SyncInfo(
                            on_wait=[w], on_update=[]
                        )
                        nc.register_instruction(nop, overwrite=True)
                        out.append(nop)
                    si.on_wait = waits[len(waits) - cap :]
                out.append(inst)
            if changed:
                bb.instructions = out


def build_nc() -> bass.Bass:
    nc = bass.Bass("TRN2", target_bir_lowering=False, debug=False)

    qT = nc.dram_tensor("qT", [E, T], BF16, kind="ExternalInput").ap()
    kT = nc.dram_tensor("kT", [E, T], BF16, kind="ExternalInput").ap()
    vT = nc.dram_tensor("vT", [E, T], BF16, kind="ExternalInput").ap()
    wq = nc.dram_tensor("wq", [E, GC], BF16, kind="ExternalInput").ap()
    wk = nc.dram_tensor("wk", [E, GC], BF16, kind="ExternalInput").ap()
    wv = nc.dram_tensor("wv", [E, GC], BF16, kind="ExternalInput").ap()
    wo = nc.dram_tensor("wo", [GC, E], BF16, kind="ExternalInput").ap()
    bq = nc.dram_tensor("bq", [GC], F32, kind="ExternalInput").ap()
    bk = nc.dram_tensor("bk", [GC], F32, kind="ExternalInput").ap()
    ident = nc.dram_tensor("ident", [P, P], BF16, kind="ExternalInput").ap()
    out = nc.dram_tensor("out", [T, E], BF16, kind="ExternalOutput").ap()

    with SplitDrainTileContext(nc) as tc:
        _build_body(nc, tc, qT, kT, vT, wq, wk, wv, wo, bq, bk, ident, out)
    _split_multi_waits(nc)
    return nc


def _build_body(nc, tc, qT, kT, vT, wq, wk, wv, wo, bq, bk, ident, out):
    from contextlib import ExitStack

    ctx = ExitStack()
    with ctx:
        cpool = ctx.enter_context(tc.tile_pool(name="consts", bufs=1))
        xpool = ctx.enter_context(tc.tile_pool(name="xstream", bufs=12))
        vpool = ctx.enter_context(tc.tile_pool(name="vstream", bufs=6))
        ptpool = ctx.enter_context(tc.tile_pool(name="pt", bufs=20))
        anpool = ctx.enter_context(tc.tile_pool(name="an", bufs=8))
        rpool = ctx.enter_context(tc.tile_pool(name="rec", bufs=4))
        opool = ctx.enter_context(tc.tile_pool(name="ostage", bufs=6))
        # PSUM: S ring 2x2 banks, qk/outproj bank, v/rowsum bank, acc 2x1.
        psS = ctx.enter_context(tc.tile_pool(name="psS", bufs=2, space="PSUM"))
        psF = ctx.enter_context(tc.tile_pool(name="psF", bufs=1, space="PSUM"))
        psV = ctx.enter_context(tc.tile_pool(name="psV", bufs=1, space="PSUM"))
        psA = ctx.enter_context(tc.tile_pool(name="psA", bufs=2, space="PSUM"))

        # ---- persistent tiles ----
        wk_sb = cpool.tile([P, KC, GC], BF16, tag="wk")
        wq_sb = cpool.tile([P, KC, GC], BF16, tag="wq")
        wv_sb = cpool.tile([P, KC, GC], BF16, tag="wv")
        bq_sb = cpool.tile([P, 2], F32, tag="bq")
        bk_sb = cpool.tile([P, 2], F32, tag="bk")
        wo_sb = cpool.tile([P, 2, E], BF16, tag="wo")
        ones = cpool.tile([P, 1], BF16, tag="ones")
        nc.vector.memset(ones[:], 1.0)
        id_sb = cpool.tile([P, P], BF16, tag="ident")

        qhB = [
            [cpool.tile([P, 512], F32R, tag=f"qh{hp}_{j}", name=f"qh{hp}_{j}")
             for j in range(NJ)]
            for hp in range(2)
        ]
        khB = [
            [cpool.tile([P, 512], F32R, tag=f"kh{hp}_{j}", name=f"kh{hp}_{j}")
             for j in range(NJ)]
            for hp in range(2)
        ]
        vh = [
            cpool.tile([P, 4, HD], BF16, tag=f"vh_{i}", name=f"vh_{i}")
            for i in range(16)
        ]
        attT = [
            [cpool.tile([P, P], BF16, tag=f"attT{kk}_{cb}", name=f"attT{kk}_{cb}")
             for cb in range(16)]
            for kk in range(2)
        ]

        # bank-5 persistent tile: v-proj region [0:256), rowsums [496:512)
        b5 = psV.tile([P, 512], F32, tag="b5", name="b5")

        def tag(bi, label):
            try:
                EMIT_LOG[bi.ins.name] = label
            except Exception:
                pass
            return bi

        # ---- emitter virtual clocks ----
        clk = {"pe": 0.0, "act": 0.0, "dve": 0.0, "pool": 0.0, "sp": 0.0}
        exp_done = {}          # s -> est completion of exp for tile s
        qh_rdy, kh_rdy, vh_rdy = {}, {}, {}
        attT_rdy = {}          # (kk, cb) -> est time
        bank4_free = [0.0]
        regC_free = [0.0]
        n_fast = [0]

        def pe(cost, ready=0.0):
            clk["pe"] = max(clk["pe"], ready) + cost
            return clk["pe"]

        # ---- DMA loads (SP queue, serial; order = priority) ----
        x_rdy = {}             # ('k'|'q'|'v', j, kh) -> est arrival
        xk, xq, xv = {}, {}, {}

        def dma_cost(nbytes, elem):
            desc = nbytes / elem
            mult = 2.0 if elem < 512 else 1.0
            return desc / 16.0 * max(elem * mult / DMA_BPN, 7.0)

        def load_w(dst, src, kh):
            nc.sync.dma_start(
                dst[:, 4 * kh : 4 * (kh + 1), :],
                src[kh * 512 : (kh + 1) * 512, :].rearrange(
                    "(kc p) c -> p kc c", p=P
                ),
            )
            clk["sp"] += dma_cost(512 * GC * 2, GC * 2)

        def load_x(kind, dram, j, kh, pool, quarters=False):
            t = pool.tile([P, 4, 512], BF16, tag="xb", name=f"x{kind}{j}{kh}")
            if quarters:
                for q2 in range(2):
                    nc.sync.dma_start(
                        t[:, 2 * q2 : 2 * (q2 + 1), :],
                        dram[
                            kh * 512 + q2 * 256 : kh * 512 + (q2 + 1) * 256,
                            j * 512 : (j + 1) * 512,
                        ].rearrange("(kc p) t -> p kc t", p=P),
                    )
                    clk["sp"] += dma_cost(256 * 512 * 2, 512 * 2)
            else:
                nc.sync.dma_start(
                    t[:],
                    dram[
                        kh * 512 : (kh + 1) * 512, j * 512 : (j + 1) * 512
                    ].rearrange("(kc p) t -> p kc t", p=P),
                )
                clk["sp"] += dma_cost(512 * 512 * 2, 512 * 2)
            x_rdy[(kind, j, kh)] = clk["sp"] + SEM
            return t

        def load_b(dst, src):
            nc.sync.dma_start(dst[:], src.rearrange("(hp p) -> p hp", p=P))
            clk["sp"] += 120.0

        # priority order: interleave k/q j0 quarter-pairs so both
        # projection streams progress as data arrives.
        def load_xq(kind, dram, j, kh, q2, t):
            nc.sync.dma_start(
                t[:, 2 * q2 : 2 * (q2 + 1), :],
                dram[
                    kh * 512 + q2 * 256 : kh * 512 + (q2 + 1) * 256,
                    j * 512 : (j + 1) * 512,
                ].rearrange("(kc p) t -> p kc t", p=P),
            )
            clk["sp"] += dma_cost(256 * 512 * 2, 512 * 2)
            x_rdy[(kind, j, kh)] = clk["sp"] + SEM

        xk[0] = [xpool.tile([P, 4, 512], BF16, tag="xb", name=f"xk0{kh}")
                 for kh in range(2)]
        xq[0] = [xpool.tile([P, 4, 512], BF16, tag="xb", name=f"xq0{kh}")
                 for kh in range(2)]
        load_w(wk_sb, wk, 0)
        load_xq("k", kT, 0, 0, 0, xk[0][0])
        load_w(wq_sb, wq, 0)
        load_xq("q", qT, 0, 0, 0, xq[0][0])
        load_xq("k", kT, 0, 0, 1, xk[0][0])
        load_xq("q", qT, 0, 0, 1, xq[0][0])
        load_b(bk_sb, bk)
        load_b(bq_sb, bq)
        load_w(wk_sb, wk, 1)
        load_xq("k", kT, 0, 1, 0, xk[0][1])
        load_w(wq_sb, wq, 1)
        load_xq("q", qT, 0, 1, 0, xq[0][1])
        load_xq("k", kT, 0, 1, 1, xk[0][1])
        load_xq("q", qT, 0, 1, 1, xq[0][1])
        nc.sync.dma_start(id_sb[:], ident)
        clk["sp"] += 120.0
        xk[1] = [load_x("k", kT, 1, kh, xpool) for kh in range(2)]
        load_w(wv_sb, wv, 0)
        load_w(wv_sb, wv, 1)
        xk[2] = [load_x("k", kT, 2, kh, xpool) for kh in range(2)]
        xv[0] = [load_x("v", vT, 0, kh, vpool) for kh in range(2)]
        xk[3] = [load_x("k", kT, 3, kh, xpool) for kh in range(2)]
        xv[1] = [load_x("v", vT, 1, kh, vpool) for kh in range(2)]
        xq[1] = [load_x("q", qT, 1, kh, xpool) for kh in range(2)]
        xv[2] = [load_x("v", vT, 2, kh, vpool) for kh in range(2)]
        nc.sync.dma_start(wo_sb[:], wo.rearrange("(kk p) e -> p kk e", p=P))
        clk["sp"] += dma_cost(GC * E * 2, E * 2)
        xv[3] = [load_x("v", vT, 3, kh, vpool) for kh in range(2)]
        xq[2] = [load_x("q", qT, 2, kh, xpool) for kh in range(2)]
        xq[3] = [load_x("q", qT, 3, kh, xpool) for kh in range(2)]

        # ---- building blocks ----
        pts = {}

        def s_step(g, i):
            j, hp = g
            S = psS.tile([P, 1024], F32, tag="S", name=f"S{j}{hp}_{i}")
            ready = max(
                exp_done.get(16 * (2 * j + hp) + i - 2, 0.0),
                kh_rdy[(hp, i // 4)],
                qh_rdy[(hp, j)],
            )
            for hb in range(2):
                cs = slice(hb * HD, (hb + 1) * HD)
                tag(nc.tensor.matmul(
                    S[:, hb * 512 : (hb + 1) * 512],
                    lhsT=khB[hp][i // 4][cs, (i % 4) * P : (i % 4 + 1) * P],
                    rhs=qhB[hp][j][cs, :],
                    start=True,
                    stop=True,
                ), f"S:{16*(2*j+hp)+i}")
            t = pe(2 * 512 * PE_ROW + 6, ready)
            pt = ptpool.tile([P, 1024], BF16, tag="pt", name=f"pt{j}{hp}_{i}")
            pts[(j, hp, i)] = pt
            return S, pt, t

        def fe_dve(pt, S, lo, hi, s):
            tag(nc.vector.tensor_scalar(
                pt[:, lo:hi].bitcast(I16),
                S[:, lo:hi],
                FE_SCALE,
                FE_BIAS,
                ALU.mult,
                ALU.add,
            ), f"feD:{s}")

        def emit_exp(g, i, S, pt, s_done, s):
            """exp engine choice per tile: whole-tile on ACT, whole-tile on
            DVE fast-exp, or split halves across both (halves the latency
            when both engines have queue room). DVE budget in half-tiles."""
            cap = FE_CAP[s // 16]
            act_full = max(clk["act"], s_done) + ACT_TILE
            dve_full = max(clk["dve"], s_done) + DVE_FE
            a_half = max(clk["act"], s_done) + 650.0
            d_half = max(clk["dve"], s_done) + 730.0
            split_fin = max(a_half, d_half)
            best = act_full
            mode = "act"
            if s < 124:
                # (last tiles stay on ACT: the epilogue drain chain needs
                # the DVE queue clear)
                if (SPLIT_EXP and n_fast[0] + 1 <= cap
                        and split_fin + 60.0 < best):
                    best, mode = split_fin, "split"
                if n_fast[0] + 2 <= cap and dve_full + 60.0 < best:
                    best, mode = dve_full, "dve"
            if mode == "act":
                tag(nc.scalar.activation(pt[:], S[:], AF.Exp, scale=SCALE),
                    f"exp:{s}")
                clk["act"] = act_full
            elif mode == "dve":
                fe_dve(pt, S, 0, 1024, s)
                clk["dve"] = dve_full
                n_fast[0] += 2
            else:
                tag(nc.scalar.activation(
                    pt[:, 0:512], S[:, 0:512], AF.Exp, scale=SCALE),
                    f"exp:{s}")
                fe_dve(pt, S, 512, 1024, s)
                clk["act"] = a_half
                clk["dve"] = d_half
                n_fast[0] += 1
            exp_done[s] = best


        def drain_op(cost_act, cost_dve, ready, emit_act, emit_dve):
            """Run a PSUM-draining op on whichever of ACT/DVE finishes
            first per the virtual clocks; returns est completion."""
            act_fin = max(clk["act"], ready) + cost_act
            dve_fin = max(clk["dve"], ready) + cost_dve
            if act_fin + 300.0 < dve_fin:
                emit_act()
                clk["act"] = act_fin
                return act_fin
            emit_dve()
            clk["dve"] = dve_fin
            return dve_fin

        def rs_word(g, r):
            par = (2 * g[0] + g[1]) % 2
            off = 496 + 8 * par + r
            return b5[:, off : off + 1]

        def av_step(g, i, acc):
            j, hp = g
            pt = pts[(j, hp, i)]
            for ch in range(4):
                for hb in range(2):
                    r = 2 * ch + hb
                    lhsT = pt[:, hb * 512 + ch * P : hb * 512 + (ch + 1) * P]
                    tag(nc.tensor.matmul(
                        acc[:, r, :],
                        lhsT=lhsT,
                        rhs=vh[i][:, 2 * hp + hb, :],
                        start=(i == 0 and r == 0),
                        stop=(i == 15),
                    ), f"av:{16*(2*j+hp)+i}")
                    nc.tensor.matmul(
                        rs_word(g, r),
                        lhsT=lhsT,
                        rhs=ones[:],
                        start=False,
                        stop=(i == 15),
                        skip_group_check=True,
                    )
            pe(
                8 * (HD + 1) * PE_ROW + 20,
                max(exp_done[16 * (2 * j + hp) + i], vh_rdy.get(i, 0.0)),
            )
            pts.pop((j, hp, i))

        pend_tr = []

        def drain_norm(g, acc):
            """Normalize group g: recip + scale (DVE) + queue PE transposes.

            The 4 chunk transposes are queued (pend_tr) and emitted into the
            following steps so PE never head-of-line blocks on the DVE
            chain. They write into the just-drained acc bank: its matmul
            accumulation group is closed, and a PE transpose only corrupts
            concurrently-open accumulations in its OWN bank.
            """
            j, hp = g
            par = (2 * j + hp) % 2
            rec = rpool.tile([P, 8], F32, tag="rec", name=f"rec{j}{hp}")
            rsv = b5[:, 496 + 8 * par : 496 + 8 * par + 8]
            nc.vector.reciprocal(rec[:], rsv)
            clk["dve"] = max(clk["dve"], clk["pe"]) + 150
            for ch in range(4):
                an = anpool.tile([P, P], BF16, tag="an", name=f"an{j}{hp}{ch}")
                for hb in range(2):
                    r = 2 * ch + hb
                    nc.vector.tensor_scalar_mul(
                        an[:, hb * HD : (hb + 1) * HD],
                        acc[:, r, :],
                        rec[:, r : r + 1],
                    )
                clk["dve"] += 2 * 200
                # re-zero the transpose slot (acc words of this chunk)
                nc.vector.memset(acc[:, 2 * ch, :], 0.0)
                clk["dve"] += 190
                pend_tr.append((g, acc, ch, an, clk["dve"]))

        def emit_transpose():
            if not pend_tr:
                return False
            g, acc, ch, an, rdy = pend_tr.pop(0)
            j, hp = g
            tsl = acc[:, 2 * ch : 2 * ch + 1, :].bitcast(BF16)
            nc.tensor.matmul(
                tsl,
                lhsT=an[:],
                rhs=id_sb[:],
                is_transpose=True,
                start=False,
                stop=True,
                skip_group_check=True,
            )
            tt = pe(56, rdy)
            dst = attT[hp][4 * j + ch]
            fin = drain_op(
                300.0, 200.0, tt + 100,
                lambda: nc.scalar.copy(out=dst[:], in_=tsl),
                lambda: nc.vector.tensor_copy(out=dst[:], in_=tsl),
            )
            attT_rdy[(hp, 4 * j + ch)] = fin + 100
            return True

        # ---- filler machinery        # ---- filler machinery (unit = one PE matmul) ----
        # Each quantum = list of (ready_fn, emit_fn); at most one open
        # quantum per PSUM bank ("4" = bank4, "C" = bank-5 region C).
        def mk_qk(kind, xh, w_sb, b_sb, dst, j, hp):
            state = {}

            def mm(kc):
                def ready():
                    r = x_rdy[(kind, j, kc // 4)]
                    if kc == 0:
                        r = max(r, bank4_free[0])
                    return r

                def fn():
                    if "ps" not in state:
                        state["ps"] = psF.tile(
                            [P, 512], F32, tag="F", name=f"p{kind}{j}{hp}"
                        )
                    ps = state["ps"]
                    tag(nc.tensor.matmul(
                        ps[:],
                        lhsT=w_sb[:, kc, hp * P : (hp + 1) * P],
                        rhs=xh[kc // 4][:, kc % 4, :],
                        start=(kc == 0),
                        stop=(kc == 7),
                    ), f"{kind}proj:{j}.{hp}.{kc}")
                    t = pe(512 * PE_ROW + 3, ready())
                    if kc == 7:
                        fin = drain_op(
                            640.0, 700.0, t + 150,
                            lambda: nc.scalar.add(
                                dst[hp][j][:], ps[:], b_sb[:, hp : hp + 1]
                            ),
                            lambda: nc.vector.tensor_scalar_add(
                                dst[hp][j][:], ps[:], b_sb[:, hp : hp + 1]
                            ),
                        )
                        (qh_rdy if kind == "q" else kh_rdy)[(hp, j)] = fin + 100
                        bank4_free[0] = fin
                return ready, fn

            return [mm(kc) for kc in range(8)]

        def mk_v(ib, ii):
            i = 4 * ib + ii
            state = {}

            def mm(kc):
                def ready():
                    r = x_rdy[("v", ib, kc // 4)]
                    if kc == 0:
                        r = max(r, regC_free[0])
                    return r

                def fn():
                    if "st" not in state:
                        state["st"] = 1
                        nc.vector.memset(b5[:, 0:GC], 0.0)
                        clk["dve"] = max(clk["dve"], regC_free[0]) + 400
                        state["ms"] = clk["dve"]
                    tag(nc.tensor.matmul(
                        b5[:, 0:GC],
                        lhsT=xv[ib][kc // 4][:, kc % 4, ii * P : (ii + 1) * P],
                        rhs=wv_sb[:, kc, :],
                        start=False,
                        stop=(kc == 7),
                        skip_group_check=True,
                    ), f"vproj:{i}.{kc}")
                    t = pe(
                        GC * PE_ROW + 3,
                        max(ready(), state["ms"] if kc == 0 else 0.0),
                    )
                    if kc == 7:
                        vsrc = b5[:, 0:GC].rearrange("p (h c) -> p h c", h=4)
                        fin = drain_op(
                            420.0, 460.0, t + 150,
                            lambda: nc.scalar.copy(out=vh[i][:], in_=vsrc),
                            lambda: nc.vector.tensor_copy(
                                out=vh[i][:], in_=vsrc
                            ),
                        )
                        vh_rdy[i] = fin + 100
                        regC_free[0] = fin
                return ready, fn

            return [mm(kc) for kc in range(8)]

        odma_n = [0]

        def emit_out_dma(cb, e0, width, ost, fin):
            odma_n[0] += 1
            cost = dma_cost(P * width * 2, width * 2)
            if odma_n[0] % 2:
                nc.gpsimd.dma_start(
                    out[cb * P : (cb + 1) * P, e0 : e0 + width], ost[:]
                )
                clk["pool"] = max(clk["pool"], fin) + cost + 700
            else:
                nc.sync.dma_start(
                    out[cb * P : (cb + 1) * P, e0 : e0 + width], ost[:]
                )
                clk["sp"] = max(clk["sp"], fin) + cost + 700

        def mk_oph(cb, e2):
            """Out-proj half [P,512] on bank4."""
            state = {}

            def mm(kk):
                def ready():
                    r = attT_rdy.get((kk, cb), INF)
                    if kk == 0:
                        r = max(r, bank4_free[0],
                                attT_rdy.get((1, cb), INF))
                    return r

                def fn():
                    if "ps" not in state:
                        state["ps"] = psF.tile(
                            [P, 512], F32, tag="F", name=f"po{cb}_{e2}"
                        )
                    ps = state["ps"]
                    tag(nc.tensor.matmul(
                        ps[:],
                        lhsT=attT[kk][cb][:],
                        rhs=wo_sb[:, kk, e2 * 512 : (e2 + 1) * 512],
                        start=(kk == 0),
                        stop=(kk == 1),
                    ), f"oph:{cb}.{e2}.{kk}")
                    t = pe(512 * PE_ROW + 3, ready())
                    if kk == 1:
                        ost = opool.tile(
                            [P, 512], BF16, tag="ost", name=f"ost{cb}_{e2}"
                        )
                        fin = drain_op(
                            700.0, 780.0, t + 150,
                            lambda: nc.scalar.copy(out=ost[:], in_=ps[:]),
                            lambda: nc.vector.tensor_copy(
                                out=ost[:], in_=ps[:]
                            ),
                        )
                        bank4_free[0] = fin
                        emit_out_dma(cb, e2 * 512, 512, ost, fin)
                return ready, fn

            return [mm(kk) for kk in range(2)]

        # filler tiles, EDF by due-step with per-bank open slots
        fillers = []

        def add_filler(due, bank, quanta):
            fillers.append({"due": due, "bank": bank, "q": quanta})

        for jb in range(1, 4):
            add_filler(4 * jb - 2, "4",
                       mk_qk("k", xk[jb], wk_sb, bk_sb, khB, jb, 0))
        add_filler(13, "4", mk_qk("k", xk[0], wk_sb, bk_sb, khB, 0, 1))
        add_filler(14, "4", mk_qk("q", xq[0], wq_sb, bq_sb, qhB, 0, 1))
        for jb in range(1, 4):
            add_filler(14 + 4 * jb - 2, "4",
                       mk_qk("k", xk[jb], wk_sb, bk_sb, khB, jb, 1))
        for ib in range(4):
            for ii in range(4):
                add_filler(6 + 4 * ib + ii, "C", mk_v(ib, ii))
        for jq in range(1, 4):
            for hp in range(2):
                add_filler(16 * (2 * jq + hp) - 5, "4",
                           mk_qk("q", xq[jq], wq_sb, bq_sb, qhB, jq, hp))
        for j in range(3):
            for ch in range(4):
                cb = 4 * j + ch
                due = 16 * (2 * j + 3) + 4 * ch + 6
                add_filler(due, "4", mk_oph(cb, 0))
                add_filler(due + 2, "4", mk_oph(cb, 1))
        fillers.sort(key=lambda f: f["due"])

        opens = {"4": None, "C": None}

        def emit_filler(horizon):
            """Emit one filler mm whose est ready <= horizon. EDF."""
            cands = []
            for b in ("4", "C"):
                if opens[b] is not None:
                    cands.append(opens[b])
                else:
                    for f in fillers:
                        if f["bank"] == b:
                            cands.append(f)
                            break
            cands = [f for f in cands if f["q"][0][0]() <= horizon]
            if not cands:
                return False
            f = min(cands, key=lambda c: c["due"])
            if opens[f["bank"]] is not f:
                fillers.remove(f)
                opens[f["bank"]] = f
            f["q"].pop(0)[1]()
            if not f["q"]:
                opens[f["bank"]] = None
            return True

        # ---- warmup: start the PE p-state ramp clock immediately on the
        # ones tile (available right after its memset), then absorb the
        # post-DMA dispatch burst with a second tiny-matmul string ----
        warm = psS.tile([P, 1024], F32, tag="S", name="warm")
        for w in range(40):
            nc.tensor.matmul(
                warm[0:1, 0:1],
                lhsT=ones[:, 0:1],
                rhs=ones[:, 0:1],
                start=(w == 0),
                stop=(w == 39),
            )
        clk["pe"] = x_rdy[("k", 0, 0)] + 100
        # second warmup string at the post-DMA-stall dispatch point: the
        # burst after a stall is priced at the LOW p-state; burn it on N=1
        # matmuls so the real projections dispatch at MID/FULL rate
        for w in range(32):
            nc.tensor.matmul(
                warm[0:8, 2:3],
                lhsT=xk[0][0][:, 0, 0:8],
                rhs=xk[0][0][:, 0, 0:1],
                start=(w == 0),
                stop=(w == 31),
            )

        # ---- prologue: k(0,0) on bank4 and q(0,0) on the bank-5 region
        # concurrently (they would otherwise serialize on one bank) ----
        kq = mk_qk("k", xk[0], wk_sb, bk_sb, khB, 0, 0)
        for kc in range(4):
            kq[kc][1]()
        # q(0,0): two 256-token half-quanta through the bank-5 C region
        for th in range(2):
            nc.vector.memset(b5[:, 0:GC], 0.0)
            clk["dve"] = max(clk["dve"], regC_free[0]) + 400
            ms = clk["dve"]
            for kc in range(8):
                tag(nc.tensor.matmul(
                    b5[:, 0:GC],
                    lhsT=wq_sb[:, kc, 0:P],
                    rhs=xq[0][kc // 4][:, kc % 4,
                                       th * 256 : (th + 1) * 256],
                    start=False,
                    stop=(kc == 7),
                    skip_group_check=True,
                ), f"qproj:0.0.{kc}")
                pe(GC * PE_ROW + 3,
                   max(x_rdy[("q", 0, kc // 4)], ms if kc == 0 else 0.0))
                if kc == 3:
                    kq[4 + th * 2][1]()
                    kq[5 + th * 2][1]()
            fin = drain_op(
                420.0, 460.0, clk["pe"] + 150,
                lambda: nc.scalar.add(
                    qhB[0][0][:, th * 256 : (th + 1) * 256],
                    b5[:, 0:GC], bq_sb[:, 0:1]),
                lambda: nc.vector.tensor_scalar_add(
                    qhB[0][0][:, th * 256 : (th + 1) * 256],
                    b5[:, 0:GC], bq_sb[:, 0:1]),
            )
            regC_free[0] = fin
        qh_rdy[(0, 0)] = fin + 100

        # ---- main loop ----
        accs = {}
        acc8 = None
        for p in range(8):
            g = GS[p]
            if p >= 1:
                gprev = GS[p - 1]
                accs[gprev] = psA.tile(
                    [P, 8, HD], F32, tag="acc", name=f"acc{p}"
                )
                # memset this group's rowsum corner (Pool) before first use
                par = (2 * gprev[0] + gprev[1]) % 2
                nc.vector.memset(b5[:, 496 + 8 * par : 504 + 8 * par], 0.0)
                clk["dve"] += 150
            if p == 7:
                acc8 = psA.tile([P, 8, HD], F32, tag="acc", name="acc8")
                par = (2 * g[0] + g[1]) % 2
                nc.vector.memset(b5[:, 496 + 8 * par : 504 + 8 * par], 0.0)
                clk["dve"] += 150
            for i in range(16):
                s = 16 * p + i
                if i >= 1:
                    emit_transpose()
                # hard prerequisites: producers must be EMITTED before use
                need = [
                    lambda: (g[1], i // 4) in kh_rdy,
                    lambda: (g[1], g[0]) in qh_rdy,
                ]
                if p >= 1:
                    need.append(lambda: i in vh_rdy)
                for cond in need:
                    while not cond():
                        # horizon below INF: never pop a quantum whose
                        # producer has not been emitted yet (ready=INF)
                        if not emit_filler(INF / 2):
                            raise RuntimeError("filler starvation")
                if p >= 1:
                    av_step(GS[p - 1], i, accs[GS[p - 1]])
                # rate-controlled fillers before this step's S matmuls
                gate = max(
                    exp_done.get(s - 2, 0.0),
                    kh_rdy[(g[1], i // 4)],
                    qh_rdy[(g[1], g[0])],
                )
                while clk["pe"] + 180 < gate - EPS:
                    if not emit_filler(gate):
                        break
                S, pt, s_done = s_step(g, i)
                emit_exp(g, i, S, pt, s_done, s)
            if p >= 1:
                acc_prev = accs.pop(GS[p - 1])
                drain_norm(GS[p - 1], acc_prev)

        # ---- epilogue: dense AV burst for the last group (its exps are
        # already done or in flight), drain(GS[6]) transposes and leftover
        # fillers interleaved, then drain(GS[7]) with per-chunk pipelined
        # out-proj j=3 through the freed S-ring banks ----
        poE = {}

        def psj3_kk0(cb):
            """Pre-start j3 out-proj with the early-available attT[0]."""
            po = psS.tile([P, 1024], F32, tag="S", name=f"poE{cb}")
            poE[cb] = po
            for e2 in range(2):
                nc.tensor.matmul(
                    po[:, e2 * 512 : (e2 + 1) * 512],
                    lhsT=attT[0][cb][:],
                    rhs=wo_sb[:, 0, e2 * 512 : (e2 + 1) * 512],
                    start=True,
                    stop=False,
                )
            pe(2 * 512 * PE_ROW + 6, attT_rdy[(0, cb)])

        def psj3(cb):
            if cb in poE:
                po = poE[cb]
                for e2 in range(2):
                    nc.tensor.matmul(
                        po[:, e2 * 512 : (e2 + 1) * 512],
                        lhsT=attT[1][cb][:],
                        rhs=wo_sb[:, 1, e2 * 512 : (e2 + 1) * 512],
                        start=False,
                        stop=True,
                    )
                t = pe(2 * 512 * PE_ROW + 6, attT_rdy[(1, cb)])
            else:
                po = psS.tile([P, 1024], F32, tag="S", name=f"poE{cb}")
                for e2 in range(2):
                    for kk in range(2):
                        nc.tensor.matmul(
                            po[:, e2 * 512 : (e2 + 1) * 512],
                            lhsT=attT[kk][cb][:],
                            rhs=wo_sb[:, kk, e2 * 512 : (e2 + 1) * 512],
                            start=(kk == 0),
                            stop=(kk == 1),
                        )
                t = pe(4 * 512 * PE_ROW + 12,
                       max(attT_rdy[(0, cb)], attT_rdy[(1, cb)]))
            ost = opool.tile([P, 1024], BF16, tag="ostE", name=f"ostE{cb}")
            if cb >= 14:
                # last chunk: halve the copy across ACT+DVE and the DMA
                # across SP+Pool to shorten the final drain chain
                nc.scalar.copy(out=ost[:, 0:512], in_=po[:, 0:512])
                nc.vector.tensor_copy(out=ost[:, 512:1024],
                                      in_=po[:, 512:1024])
                clk["act"] = max(clk["act"], t + 150) + 650
                clk["dve"] = max(clk["dve"], t + 150) + 700
                nc.sync.dma_start(out[cb * P : (cb + 1) * P, 0:512],
                                  ost[:, 0:512])
                if cb == 14:
                    nc.gpsimd.dma_start(
                        out[cb * P : (cb + 1) * P, 512:1024],
                        ost[:, 512:1024])
                else:
                    nc.sync.dma_start(
                        out[cb * P : (cb + 1) * P, 512:1024],
                        ost[:, 512:1024])
                return
            fin = drain_op(
                1230.0, 1250.0, t + 150,
                lambda: nc.scalar.copy(out=ost[:], in_=po[:]),
                lambda: nc.vector.tensor_copy(out=ost[:], in_=po[:]),
            )
            if cb == 13:
                nc.gpsimd.dma_start(out[cb * P : (cb + 1) * P, :], ost[:])
                clk["pool"] = max(clk["pool"], fin) + 900
            else:
                nc.sync.dma_start(out[cb * P : (cb + 1) * P, :], ost[:])
                clk["sp"] = max(clk["sp"], fin) + 900

        for i in range(16):
            av_step(GS[7], i, acc8)
            emit_transpose()
            emit_filler(INF / 2)
        drain_norm(GS[7], acc8)
        # psS has 2 buffers: pre-start the first two chunks on attT[0]
        psj3_kk0(12)
        psj3_kk0(13)
        while fillers or opens["4"] is not None or opens["C"] is not None:
            if not emit_filler(INF / 2):
                break
        for ch in range(4):
            emit_transpose()
            psj3(12 + ch)
        while emit_transpose():
            pass


_NC_CACHE: list = []


def kernel(q, k, v, Wq, bq, Wk, bk, Wv, bv, Wo, bo):
    q = np.asarray(q, dtype=np.float32)
    k = np.asarray(k, dtype=np.float32)
    v = np.asarray(v, dtype=np.float32)
    Wq = np.asarray(Wq, dtype=np.float32)
    Wk = np.asarray(Wk, dtype=np.float32)
    Wv = np.asarray(Wv, dtype=np.float32)
    Wo = np.asarray(Wo, dtype=np.float32)
    bq = np.asarray(bq, dtype=np.float32)
    bk = np.asarray(bk, dtype=np.float32)
    bv = np.asarray(bv, dtype=np.float32)
    bo = np.asarray(bo, dtype=np.float32)

    if not _NC_CACHE:
        _NC_CACHE.append(build_nc())
    nc = _NC_CACHE[0]

    bf = ml_dtypes.bfloat16
    qTb = [np.ascontiguousarray(q[b].T).astype(bf) for b in range(B)]
    kTb = [np.ascontiguousarray(k[b].T).astype(bf) for b in range(B)]
    vTb = [np.ascontiguousarray(v[b].T).astype(bf) for b in range(B)]

    in_maps = []
    for c in range(NC_):
        b, g = divmod(c, GROUPS)
        cs = slice(g * GC, (g + 1) * GC)
        in_maps.append(
            {
                "qT": qTb[b],
                "kT": kTb[b],
                "vT": vTb[b],
                "wq": Wq[:, cs].astype(bf),
                "wk": Wk[:, cs].astype(bf),
                "wv": Wv[:, cs].astype(bf),
                "wo": np.ascontiguousarray(Wo[cs, :]).astype(bf),
                "bq": bq[cs],
                "bk": bk[cs],
                "ident": np.eye(P, dtype=bf),
            }
        )

    kw = {}
    if TRACE:
        kw = dict(trace=True, tmpdir=TRACE_DIR, **TRACE_KW)
    res = run_bass_kernel_spmd(nc, in_maps, core_ids=list(range(NC_)), **kw)
    LAST_RESULT.clear()
    LAST_RESULT.append(res)

    outp = np.zeros((B, T, E), dtype=np.float32)
    for c in range(NC_):
        b = c // GROUPS
        outp[b] += res.results[c]["out"].astype(np.float32)
    # bv's contribution (softmax rows sum to 1): (1 . bv^T) @ Wo, plus bo
    outp += bo + bv @ Wo
    return outp


TRACE = False
TRACE_DIR = None
TRACE_KW: dict = {}
LAST_RESULT: list = []
EMIT_LOG: dict = {}


# revision 38
# speedup vs baseline: 1.0113x; 1.0113x over previous
"""Multi-head attention (nn_Attention_987842478290) on 8 TRN2 NeuronCores.

Sharding: batch (2) x head-group (4 groups of 4 heads) = 8 cores; the host
pre-transposes q/k/v per batch and slices Wq/Wk/Wv columns (and Wo rows)
per head group, so every core runs the identical SPMD program on its shard.

Per core (one batch, 4 heads as 2 "hp" pairs):
  - q/k/v projections on PE (bf16 in, fp32 PSUM); qh/kh kept TRANSPOSED
    [head-cols, tokens] in f32r.
  - scores per (group, m-tile): S = [m128, headA-n512 | headB-n512], two
    K=64 matmuls into a dedicated 2-deep PSUM ring (banks 0-3).
  - exp: ACT (exp, scale=1/8 folded) or DVE fast-exp -- a bf16 bit-trick
    (tensor_scalar mult+add into an int16 view of the pt tile, ~1.8% rms
    error, bias-centered) -- chosen greedily per tile by estimated engine
    completion so the exp stream never paces PE; DVE share capped for
    numerics (~45% measured, rel err 8.4e-3 vs 2e-2 budget).
  - AV pt-stationary: acc[(chunk,hb), c64] accumulated over 16 m-tiles in
    ONE PSUM bank per group (banks 6-7 alternate); softmax row-sums are
    accumulated by 1-row ones-matmuls into a 16-word corner of bank 5
    (start=False + DVE memset: a bank-wide start=True zero-region would
    clobber co-tenants).
  - normalization: DVE reciprocal + per-partition scalar multiplies, then
    a PE transpose matmul (is_transpose, identity moving operand) into a
    re-zeroed slot of the just-drained acc bank, staged to SBUF attT by an
    ACT/DVE copy. NOTE: a PE transpose corrupts any concurrently-OPEN
    accumulation group in its own PSUM bank (walrus/birsim behavior), so
    transposes only ever target banks whose groups are all closed; they
    are queued at phase end and emitted one per step of the next phase.
  - out-projection per 128-token chunk: mid-kernel via bank-4 [P,512]
    halves (PE) staged bf16 by ACT/DVE copies, DMA'd out from alternating
    SP/Pool queues (one queue serializes: each DMA holds its issuing
    engine ~2.3us); the last j-block runs in the epilogue through the
    freed S-ring banks as full-width tiles.

Scheduling: a single-pass emitter with per-engine virtual clocks. Filler
matmul quanta (projection / out-projection single matmuls) are
rate-controlled between the S-tile matmuls under an EDF policy with
per-bank open slots and readiness estimates, so PE stays dense while the
exp stream consumes the S ring; prerequisites are force-emitted (with a
finite horizon so never-emitted producers are not queued early, which
would head-of-line block the PE). Startup interleaves j0 k/q quarter DMA
loads with both prologue projection streams on separate PSUM banks; the
epilogue runs the last group's AV as a dense burst, drains with low
latency PE transposes, and pipelines the final out-proj per chunk.

Host: sums the 8 partial outputs per batch and adds bo + bv @ Wo.
TimelineSim: 175.5us (baseline 207.1us). Rel err vs fp32 ref: 8.4e-3.
"""

import math

import numpy as np
import ml_dtypes

import concourse.bass as bass
import concourse.mybir as mybir
import concourse.tile as tile
from concourse.bass_utils import run_bass_kernel_spmd
from concourse.vector_clock import ScopedClock

F32 = mybir.dt.float32
F32R = mybir.dt.float32r
BF16 = mybir.dt.bfloat16
I16 = mybir.dt.int16
AF = mybir.ActivationFunctionType
ALU = mybir.AluOpType

B, T, E = 2, 2048, 1024
HEADS, HD = 16, 64
NC_ = 8
GROUPS = 4                  # head-groups (4 heads each)
GC = 256                    # cols per core = 4 heads * 64
P = 128
KC = E // P                 # 8 contraction chunks for projections
NJ = T // 512               # 4 n-chunks of 512
SCALE = 1.0 / np.sqrt(HD)   # 1/8
GS = [(j, hp) for j in range(NJ) for hp in range(2)]  # group sequence

# fast-exp (DVE) constants: bf16 bits = trunc(x*scale*128*log2e + bias)
FE_SCALE = float(SCALE * 128.0 / math.log(2.0))
FE_BIAS = 16248.9
MAX_FE_HALF = 160            # numerics cap, in half-tiles
# cumulative fast-exp budget by phase (half-tiles): save quota for tails
FE_CAP = [20, 40, 60, 80, 100, 120, 140, 160]

# cost-model constants for the emitter's virtual clocks (ns)
PE_ROW = 0.4167
ACT_TILE = 1105.0           # one [128,1024] exp incl per-inst overheads
DVE_FE = 1260.0             # fast-exp tensor_scalar on DVE
DMA_BPN = 22.4              # DMA bytes/ns per engine (x16 engines)
SEM = 900.0                 # DMA sem propagation
EPS = 40.0
INF = 1e18
SPLIT_EXP = False


class SplitDrainTileContext(tile.TileContext):
    """TileContext whose final drain never carries >1 sem wait.

    This walrus build rejects >1 sync-wait per instruction; the stock
    epilogue funnels every outstanding wait onto one SP Drain. Emit the
    extra waits on individual SP nops instead.
    """

    def _drain_and_barrier(self, tick_clock, wait_clock):
        drain_inst = self.nc.sync.drain()
        wait_clock.add_sem_waits(
            drain_inst.ins, ScopedClock({None: tick_clock.global_clock})
        )
        si = drain_inst.ins.sync_info
        waits = list(si.on_wait) if si is not None else []
        if len(waits) > 1:
            import bass_rust

            si.on_wait = waits[:1]
            for w in waits[1:]:
                nop = self.nc.sync.nop(nofuse=True)
                nop.ins.sync_info = bass_rust.SyncInfo(on_wait=[w], on_update=[])

        self.nc.all_engine_barrier()
        assert self.sems is not None
        popped = self.nc._tile_sem_poison_stack.pop()
        assert popped is self._sem_poison
        self.nc.clear_and_free_semaphores(list(self.sems.allocated().values()))
        self.nc.all_engine_barrier()


def _split_multi_waits(nc):
    """Move excess sem waits onto preceding same-engine nops.

    This walrus build accepts at most one sync wait per instruction (two
    for EventSemaphore); Tile's scheduler sometimes attaches more (final
    drain, DMA WAR chains). Each engine executes its block instructions
    in list order, so a nop carrying the extra wait immediately before
    the instruction preserves semantics.
    """
    import bass_rust

    for f in nc.m.functions:
        for bb in f.blocks:
            insts = list(bb.instructions)
            out, changed = [], False
            for inst in insts:
                si = inst.sync_info
                waits = list(si.on_wait) if si is not None else []
                cap = 2 if isinstance(inst, mybir.InstEventSemaphore) else 1
                if len(waits) > cap:
                    changed = True
                    for w in waits[: len(waits) - cap]:
                        nop = mybir.InstNoOp(
                            name=f"I-splitw-{nc.next_id()}",
                            ins=[],
                            outs=[],
                        )
                        nop.engine = inst.engine
                        nop.sync_info = bass_rust.SyncInfo(
                            on_wait=[w], on_update=[]
                        )
                        nc.register_instruction(nop, overwrite=True)
                        out.append(nop)
                    si.on_wait = waits[len(waits) - cap :]
                out.append(inst)
            if changed:
                bb.instructions = out


def build_nc() -> bass.Bass:
    nc = bass.Bass("TRN2", target_bir_lowering=False, debug=False)

    qT = nc.dram_tensor("qT", [E, T], BF16, kind="ExternalInput").ap()
    kT = nc.dram_tensor("kT", [E, T], BF16, kind="ExternalInput").ap()
    vT = nc.dram_tensor("vT", [E, T], BF16, kind="ExternalInput").ap()
    wq = nc.dram_tensor("wq", [E, GC], BF16, kind="ExternalInput").ap()
    wk = nc.dram_tensor("wk", [E, GC], BF16, kind="ExternalInput").ap()
    wv = nc.dram_tensor("wv", [E, GC], BF16, kind="ExternalInput").ap()
    wo = nc.dram_tensor("wo", [GC, E], BF16, kind="ExternalInput").ap()
    bq = nc.dram_tensor("bq", [GC], F32, kind="ExternalInput").ap()
    bk = nc.dram_tensor("bk", [GC], F32, kind="ExternalInput").ap()
    ident = nc.dram_tensor("ident", [P, P], BF16, kind="ExternalInput").ap()
    out = nc.dram_tensor("out", [T, E], BF16, kind="ExternalOutput").ap()

    with SplitDrainTileContext(nc) as tc:
        _build_body(nc, tc, qT, kT, vT, wq, wk, wv, wo, bq, bk, ident, out)
    _split_multi_waits(nc)
    return nc


def _build_body(nc, tc, qT, kT, vT, wq, wk, wv, wo, bq, bk, ident, out):
    from contextlib import ExitStack

    ctx = ExitStack()
    with ctx:
        cpool = ctx.enter_context(tc.tile_pool(name="consts", bufs=1))
        xpool = ctx.enter_context(tc.tile_pool(name="xstream", bufs=12))
        vpool = ctx.enter_context(tc.tile_pool(name="vstream", bufs=6))
        ptpool = ctx.enter_context(tc.tile_pool(name="pt", bufs=20))
        anpool = ctx.enter_context(tc.tile_pool(name="an", bufs=8))
        rpool = ctx.enter_context(tc.tile_pool(name="rec", bufs=4))
        opool = ctx.enter_context(tc.tile_pool(name="ostage", bufs=6))
        # PSUM: S ring 2x2 banks, qk/outproj bank, v/rowsum bank, acc 2x1.
        psS = ctx.enter_context(tc.tile_pool(name="psS", bufs=2, space="PSUM"))
        psF = ctx.enter_context(tc.tile_pool(name="psF", bufs=1, space="PSUM"))
        psV = ctx.enter_context(tc.tile_pool(name="psV", bufs=1, space="PSUM"))
        psA = ctx.enter_context(tc.tile_pool(name="psA", bufs=2, space="PSUM"))

        # ---- persistent tiles ----
        wk_sb = cpool.tile([P, KC, GC], BF16, tag="wk")
        wq_sb = cpool.tile([P, KC, GC], BF16, tag="wq")
        wv_sb = cpool.tile([P, KC, GC], BF16, tag="wv")
        bq_sb = cpool.tile([P, 2], F32, tag="bq")
        bk_sb = cpool.tile([P, 2], F32, tag="bk")
        wo_sb = cpool.tile([P, 2, E], BF16, tag="wo")
        ones = cpool.tile([P, 1], BF16, tag="ones")
        nc.vector.memset(ones[:], 1.0)
        id_sb = cpool.tile([P, P], BF16, tag="ident")

        qhB = [
            [cpool.tile([P, 512], F32R, tag=f"qh{hp}_{j}", name=f"qh{hp}_{j}")
             for j in range(NJ)]
            for hp in range(2)
        ]
        khB = [
            [cpool.tile([P, 512], F32R, tag=f"kh{hp}_{j}", name=f"kh{hp}_{j}")
             for j in range(NJ)]
            for hp in range(2)
        ]
        vh = [
            cpool.tile([P, 4, HD], BF16, tag=f"vh_{i}", name=f"vh_{i}")
            for i in range(16)
        ]
        attT = [
            [cpool.tile([P, P], BF16, tag=f"attT{kk}_{cb}", name=f"attT{kk}_{cb}")
             for cb in range(16)]
            for kk in range(2)
        ]

        # bank-5 persistent tile: v-proj region [0:256), rowsums [496:512)
        b5 = psV.tile([P, 512], F32, tag="b5", name="b5")

        def tag(bi, label):
            try:
                EMIT_LOG[bi.ins.name] = label
            except Exception:
                pass
            return bi

        # ---- emitter virtual clocks ----
        clk = {"pe": 0.0, "act": 0.0, "dve": 0.0, "pool": 0.0, "sp": 0.0}
        exp_done = {}          # s -> est completion of exp for tile s
        qh_rdy, kh_rdy, vh_rdy = {}, {}, {}
        attT_rdy = {}          # (kk, cb) -> est time
        bank4_free = [0.0]
        regC_free = [0.0]
        n_fast = [0]

        def pe(cost, ready=0.0):
            clk["pe"] = max(clk["pe"], ready) + cost
            return clk["pe"]

        # ---- DMA loads (SP queue, serial; order = priority) ----
        x_rdy = {}             # ('k'|'q'|'v', j, kh) -> est arrival
        xk, xq, xv = {}, {}, {}

        def dma_cost(nbytes, elem):
            desc = nbytes / elem
            mult = 2.0 if elem < 512 else 1.0
            return desc / 16.0 * max(elem * mult / DMA_BPN, 7.0)

        def load_w(dst, src, kh):
            nc.sync.dma_start(
                dst[:, 4 * kh : 4 * (kh + 1), :],
                src[kh * 512 : (kh + 1) * 512, :].rearrange(
                    "(kc p) c -> p kc c", p=P
                ),
            )
            clk["sp"] += dma_cost(512 * GC * 2, GC * 2)

        def load_x(kind, dram, j, kh, pool, quarters=False):
            t = pool.tile([P, 4, 512], BF16, tag="xb", name=f"x{kind}{j}{kh}")
            if quarters:
                for q2 in range(2):
                    nc.sync.dma_start(
                        t[:, 2 * q2 : 2 * (q2 + 1), :],
                        dram[
                            kh * 512 + q2 * 256 : kh * 512 + (q2 + 1) * 256,
                            j * 512 : (j + 1) * 512,
                        ].rearrange("(kc p) t -> p kc t", p=P),
                    )
                    clk["sp"] += dma_cost(256 * 512 * 2, 512 * 2)
            else:
                nc.sync.dma_start(
                    t[:],
                    dram[
                        kh * 512 : (kh + 1) * 512, j * 512 : (j + 1) * 512
                    ].rearrange("(kc p) t -> p kc t", p=P),
                )
                clk["sp"] += dma_cost(512 * 512 * 2, 512 * 2)
            x_rdy[(kind, j, kh)] = clk["sp"] + SEM
            return t

        def load_b(dst, src):
            nc.sync.dma_start(dst[:], src.rearrange("(hp p) -> p hp", p=P))
            clk["sp"] += 120.0

        # priority order: interleave k/q j0 quarter-pairs so both
        # projection streams progress as data arrives.
        def load_xq(kind, dram, j, kh, q2, t):
            nc.sync.dma_start(
                t[:, 2 * q2 : 2 * (q2 + 1), :],
                dram[
                    kh * 512 + q2 * 256 : kh * 512 + (q2 + 1) * 256,
                    j * 512 : (j + 1) * 512,
                ].rearrange("(kc p) t -> p kc t", p=P),
            )
            clk["sp"] += dma_cost(256 * 512 * 2, 512 * 2)
            x_rdy[(kind, j, kh)] = clk["sp"] + SEM

        xk[0] = [xpool.tile([P, 4, 512], BF16, tag="xb", name=f"xk0{kh}")
                 for kh in range(2)]
        xq[0] = [xpool.tile([P, 4, 512], BF16, tag="xb", name=f"xq0{kh}")
                 for kh in range(2)]
        load_w(wk_sb, wk, 0)
        load_xq("k", kT, 0, 0, 0, xk[0][0])
        load_w(wq_sb, wq, 0)
        load_xq("q", qT, 0, 0, 0, xq[0][0])
        load_xq("k", kT, 0, 0, 1, xk[0][0])
        load_xq("q", qT, 0, 0, 1, xq[0][0])
        load_b(bk_sb, bk)
        load_b(bq_sb, bq)
        load_w(wk_sb, wk, 1)
        load_xq("k", kT, 0, 1, 0, xk[0][1])
        load_w(wq_sb, wq, 1)
        load_xq("q", qT, 0, 1, 0, xq[0][1])
        load_xq("k", kT, 0, 1, 1, xk[0][1])
        load_xq("q", qT, 0, 1, 1, xq[0][1])
        nc.sync.dma_start(id_sb[:], ident)
        clk["sp"] += 120.0
        xk[1] = [load_x("k", kT, 1, kh, xpool) for kh in range(2)]
        load_w(wv_sb, wv, 0)
        load_w(wv_sb, wv, 1)
        xk[2] = [load_x("k", kT, 2, kh, xpool) for kh in range(2)]
        xv[0] = [load_x("v", vT, 0, kh, vpool) for kh in range(2)]
        xk[3] = [load_x("k", kT, 3, kh, xpool) for kh in range(2)]
        xv[1] = [load_x("v", vT, 1, kh, vpool) for kh in range(2)]
        xq[1] = [load_x("q", qT, 1, kh, xpool) for kh in range(2)]
        xv[2] = [load_x("v", vT, 2, kh, vpool) for kh in range(2)]
        nc.sync.dma_start(wo_sb[:], wo.rearrange("(kk p) e -> p kk e", p=P))
        clk["sp"] += dma_cost(GC * E * 2, E * 2)
        xv[3] = [load_x("v", vT, 3, kh, vpool) for kh in range(2)]
        xq[2] = [load_x("q", qT, 2, kh, xpool) for kh in range(2)]
        xq[3] = [load_x("q", qT, 3, kh, xpool) for kh in range(2)]

        # ---- building blocks ----
        pts = {}

        def s_step(g, i):
            j, hp = g
            S = psS.tile([P, 1024], F32, tag="S", name=f"S{j}{hp}_{i}")
            ready = max(
                exp_done.get(16 * (2 * j + hp) + i - 2, 0.0),
                kh_rdy[(hp, i // 4)],
                qh_rdy[(hp, j)],
            )
            for hb in range(2):
                cs = slice(hb * HD, (hb + 1) * HD)
                tag(nc.tensor.matmul(
                    S[:, hb * 512 : (hb + 1) * 512],
                    lhsT=khB[hp][i // 4][cs, (i % 4) * P : (i % 4 + 1) * P],
                    rhs=qhB[hp][j][cs, :],
                    start=True,
                    stop=True,
                ), f"S:{16*(2*j+hp)+i}")
            t = pe(2 * 512 * PE_ROW + 6, ready)
            pt = ptpool.tile([P, 1024], BF16, tag="pt", name=f"pt{j}{hp}_{i}")
            pts[(j, hp, i)] = pt
            return S, pt, t

        def fe_dve(pt, S, lo, hi, s):
            tag(nc.vector.tensor_scalar(
                pt[:, lo:hi].bitcast(I16),
                S[:, lo:hi],
                FE_SCALE,
                FE_BIAS,
                ALU.mult,
                ALU.add,
            ), f"feD:{s}")

        def emit_exp(g, i, S, pt, s_done, s):
            """exp engine choice per tile: whole-tile on ACT, whole-tile on
            DVE fast-exp, or split halves across both (halves the latency
            when both engines have queue room). DVE budget in half-tiles."""
            cap = FE_CAP[s // 16]
            act_full = max(clk["act"], s_done) + ACT_TILE
            dve_full = max(clk["dve"], s_done) + DVE_FE
            a_half = max(clk["act"], s_done) + 650.0
            d_half = max(clk["dve"], s_done) + 730.0
            split_fin = max(a_half, d_half)
            best = act_full
            mode = "act"
            if s < 126:
                # (last tiles stay on ACT: the epilogue drain chain needs
                # the DVE queue clear)
                if (SPLIT_EXP and n_fast[0] + 1 <= cap
                        and split_fin + 60.0 < best):
                    best, mode = split_fin, "split"
                if n_fast[0] + 2 <= cap and dve_full + 60.0 < best:
                    best, mode = dve_full, "dve"
            if mode == "act":
                tag(nc.scalar.activation(pt[:], S[:], AF.Exp, scale=SCALE),
                    f"exp:{s}")
                clk["act"] = act_full
            elif mode == "dve":
                fe_dve(pt, S, 0, 1024, s)
                clk["dve"] = dve_full
                n_fast[0] += 2
            else:
                tag(nc.scalar.activation(
                    pt[:, 0:512], S[:, 0:512], AF.Exp, scale=SCALE),
                    f"exp:{s}")
                fe_dve(pt, S, 512, 1024, s)
                clk["act"] = a_half
                clk["dve"] = d_half
                n_fast[0] += 1
            exp_done[s] = best


        def drain_op(cost_act, cost_dve, ready, emit_act, emit_dve):
            """Run a PSUM-draining op on whichever of ACT/DVE finishes
            first per the virtual clocks; returns est completion."""
            act_fin = max(clk["act"], ready) + cost_act
            dve_fin = max(clk["dve"], ready) + cost_dve
            if act_fin + 300.0 < dve_fin:
                emit_act()
                clk["act"] = act_fin
                return act_fin
            emit_dve()
            clk["dve"] = dve_fin
            return dve_fin

        def rs_word(g, r):
            par = (2 * g[0] + g[1]) % 2
            off = 496 + 8 * par + r
            return b5[:, off : off + 1]

        def av_step(g, i, acc):
            j, hp = g
            pt = pts[(j, hp, i)]
            for ch in range(4):
                for hb in range(2):
                    r = 2 * ch + hb
                    lhsT = pt[:, hb * 512 + ch * P : hb * 512 + (ch + 1) * P]
                    tag(nc.tensor.matmul(
                        acc[:, r, :],
                        lhsT=lhsT,
                        rhs=vh[i][:, 2 * hp + hb, :],
                        start=(i == 0 and r == 0),
                        stop=(i == 15),
                    ), f"av:{16*(2*j+hp)+i}")
                    nc.tensor.matmul(
                        rs_word(g, r),
                        lhsT=lhsT,
                        rhs=ones[:],
                        start=False,
                        stop=(i == 15),
                        skip_group_check=True,
                    )
            pe(
                8 * (HD + 1) * PE_ROW + 20,
                max(exp_done[16 * (2 * j + hp) + i], vh_rdy.get(i, 0.0)),
            )
            pts.pop((j, hp, i))

        pend_tr = []

        def drain_norm(g, acc):
            """Normalize group g: recip + scale (DVE) + queue PE transposes.

            The 4 chunk transposes are queued (pend_tr) and emitted into the
            following steps so PE never head-of-line blocks on the DVE
            chain. They write into the just-drained acc bank: its matmul
            accumulation group is closed, and a PE transpose only corrupts
            concurrently-open accumulations in its OWN bank.
            """
            j, hp = g
            par = (2 * j + hp) % 2
            rec = rpool.tile([P, 8], F32, tag="rec", name=f"rec{j}{hp}")
            rsv = b5[:, 496 + 8 * par : 496 + 8 * par + 8]
            nc.vector.reciprocal(rec[:], rsv)
            clk["dve"] = max(clk["dve"], clk["pe"]) + 150
            for ch in range(4):
                an = anpool.tile([P, P], BF16, tag="an", name=f"an{j}{hp}{ch}")
                for hb in range(2):
                    r = 2 * ch + hb
                    nc.vector.tensor_scalar_mul(
                        an[:, hb * HD : (hb + 1) * HD],
                        acc[:, r, :],
                        rec[:, r : r + 1],
                    )
                clk["dve"] += 2 * 200
                # re-zero the transpose slot (acc words of this chunk)
                nc.vector.memset(acc[:, 2 * ch, :], 0.0)
                clk["dve"] += 190
                pend_tr.append((g, acc, ch, an, clk["dve"]))

        def emit_transpose():
            if not pend_tr:
                return False
            g, acc, ch, an, rdy = pend_tr.pop(0)
            j, hp = g
            tsl = acc[:, 2 * ch : 2 * ch + 1, :].bitcast(BF16)
            nc.tensor.matmul(
                tsl,
                lhsT=an[:],
                rhs=id_sb[:],
                is_transpose=True,
                start=False,
                stop=True,
                skip_group_check=True,
            )
            tt = pe(56, rdy)
            dst = attT[hp][4 * j + ch]
            fin = drain_op(
                300.0, 200.0, tt + 100,
                lambda: nc.scalar.copy(out=dst[:], in_=tsl),
                lambda: nc.vector.tensor_copy(out=dst[:], in_=tsl),
            )
            attT_rdy[(hp, 4 * j + ch)] = fin + 100
            return True

        # ---- filler machinery        # ---- filler machinery (unit = one PE matmul) ----
        # Each quantum = list of (ready_fn, emit_fn); at most one open
        # quantum per PSUM bank ("4" = bank4, "C" = bank-5 region C).
        def mk_qk(kind, xh, w_sb, b_sb, dst, j, hp):
            state = {}

            def mm(kc):
                def ready():
                    r = x_rdy[(kind, j, kc // 4)]
                    if kc == 0:
                        r = max(r, bank4_free[0])
                    return r

                def fn():
                    if "ps" not in state:
                        state["ps"] = psF.tile(
                            [P, 512], F32, tag="F", name=f"p{kind}{j}{hp}"
                        )
                    ps = state["ps"]
                    tag(nc.tensor.matmul(
                        ps[:],
                        lhsT=w_sb[:, kc, hp * P : (hp + 1) * P],
                        rhs=xh[kc // 4][:, kc % 4, :],
                        start=(kc == 0),
                        stop=(kc == 7),
                    ), f"{kind}proj:{j}.{hp}.{kc}")
                    t = pe(512 * PE_ROW + 3, ready())
                    if kc == 7:
                        fin = drain_op(
                            640.0, 700.0, t + 150,
                            lambda: nc.scalar.add(
                                dst[hp][j][:], ps[:], b_sb[:, hp : hp + 1]
                            ),
                            lambda: nc.vector.tensor_scalar_add(
                                dst[hp][j][:], ps[:], b_sb[:, hp : hp + 1]
                            ),
                        )
                        (qh_rdy if kind == "q" else kh_rdy)[(hp, j)] = fin + 100
                        bank4_free[0] = fin
                return ready, fn

            return [mm(kc) for kc in range(8)]

        def mk_v(ib, ii):
            i = 4 * ib + ii
            state = {}

            def mm(kc):
                def ready():
                    r = x_rdy[("v", ib, kc // 4)]
                    if kc == 0:
                        r = max(r, regC_free[0])
                    return r

                def fn():
                    if "st" not in state:
                        state["st"] = 1
                        nc.vector.memset(b5[:, 0:GC], 0.0)
                        clk["dve"] = max(clk["dve"], regC_free[0]) + 400
                        state["ms"] = clk["dve"]
                    tag(nc.tensor.matmul(
                        b5[:, 0:GC],
                        lhsT=xv[ib][kc // 4][:, kc % 4, ii * P : (ii + 1) * P],
                        rhs=wv_sb[:, kc, :],
                        start=False,
                        stop=(kc == 7),
                        skip_group_check=True,
                    ), f"vproj:{i}.{kc}")
                    t = pe(
                        GC * PE_ROW + 3,
                        max(ready(), state["ms"] if kc == 0 else 0.0),
                    )
                    if kc == 7:
                        vsrc = b5[:, 0:GC].rearrange("p (h c) -> p h c", h=4)
                        fin = drain_op(
                            420.0, 460.0, t + 150,
                            lambda: nc.scalar.copy(out=vh[i][:], in_=vsrc),
                            lambda: nc.vector.tensor_copy(
                                out=vh[i][:], in_=vsrc
                            ),
                        )
                        vh_rdy[i] = fin + 100
                        regC_free[0] = fin
                return ready, fn

            return [mm(kc) for kc in range(8)]

        odma_n = [0]

        def emit_out_dma(cb, e0, width, ost, fin):
            odma_n[0] += 1
            cost = dma_cost(P * width * 2, width * 2)
            if odma_n[0] % 2:
                nc.gpsimd.dma_start(
                    out[cb * P : (cb + 1) * P, e0 : e0 + width], ost[:]
                )
                clk["pool"] = max(clk["pool"], fin) + cost + 700
            else:
                nc.sync.dma_start(
                    out[cb * P : (cb + 1) * P, e0 : e0 + width], ost[:]
                )
                clk["sp"] = max(clk["sp"], fin) + cost + 700

        def mk_oph(cb, e2):
            """Out-proj half [P,512] on bank4."""
            state = {}

            def mm(kk):
                def ready():
                    r = attT_rdy.get((kk, cb), INF)
                    if kk == 0:
                        r = max(r, bank4_free[0],
                                attT_rdy.get((1, cb), INF))
                    return r

                def fn():
                    if "ps" not in state:
                        state["ps"] = psF.tile(
                            [P, 512], F32, tag="F", name=f"po{cb}_{e2}"
                        )
                    ps = state["ps"]
                    tag(nc.tensor.matmul(
                        ps[:],
                        lhsT=attT[kk][cb][:],
                        rhs=wo_sb[:, kk, e2 * 512 : (e2 + 1) * 512],
                        start=(kk == 0),
                        stop=(kk == 1),
                    ), f"oph:{cb}.{e2}.{kk}")
                    t = pe(512 * PE_ROW + 3, ready())
                    if kk == 1:
                        ost = opool.tile(
                            [P, 512], BF16, tag="ost", name=f"ost{cb}_{e2}"
                        )
                        fin = drain_op(
                            700.0, 780.0, t + 150,
                            lambda: nc.scalar.copy(out=ost[:], in_=ps[:]),
                            lambda: nc.vector.tensor_copy(
                                out=ost[:], in_=ps[:]
                            ),
                        )
                        bank4_free[0] = fin
                        emit_out_dma(cb, e2 * 512, 512, ost, fin)
                return ready, fn

            return [mm(kk) for kk in range(2)]

        # filler tiles, EDF by due-step with per-bank open slots
        fillers = []

        def add_filler(due, bank, quanta):
            fillers.append({"due": due, "bank": bank, "q": quanta})

        for jb in range(1, 4):
            add_filler(4 * jb - 2, "4",
                       mk_qk("k", xk[jb], wk_sb, bk_sb, khB, jb, 0))
        add_filler(13, "4", mk_qk("k", xk[0], wk_sb, bk_sb, khB, 0, 1))
        add_filler(14, "4", mk_qk("q", xq[0], wq_sb, bq_sb, qhB, 0, 1))
        for jb in range(1, 4):
            add_filler(14 + 4 * jb - 2, "4",
                       mk_qk("k", xk[jb], wk_sb, bk_sb, khB, jb, 1))
        for ib in range(4):
            for ii in range(4):
                add_filler(6 + 4 * ib + ii, "C", mk_v(ib, ii))
        for jq in range(1, 4):
            for hp in range(2):
                add_filler(16 * (2 * jq + hp) - 5, "4",
                           mk_qk("q", xq[jq], wq_sb, bq_sb, qhB, jq, hp))
        for j in range(3):
            for ch in range(4):
                cb = 4 * j + ch
                due = 16 * (2 * j + 3) + 4 * ch + 6
                add_filler(due, "4", mk_oph(cb, 0))
                add_filler(due + 2, "4", mk_oph(cb, 1))
        fillers.sort(key=lambda f: f["due"])

        opens = {"4": None, "C": None}

        def emit_filler(horizon):
            """Emit one filler mm whose est ready <= horizon. EDF."""
            cands = []
            for b in ("4", "C"):
                if opens[b] is not None:
                    cands.append(opens[b])
                else:
                    for f in fillers:
                        if f["bank"] == b:
                            cands.append(f)
                            break
            cands = [f for f in cands if f["q"][0][0]() <= horizon]
            if not cands:
                return False
            f = min(cands, key=lambda c: c["due"])
            if opens[f["bank"]] is not f:
                fillers.remove(f)
                opens[f["bank"]] = f
            f["q"].pop(0)[1]()
            if not f["q"]:
                opens[f["bank"]] = None
            return True

        # ---- warmup: start the PE p-state ramp clock immediately on the
        # ones tile (available right after its memset), then absorb the
        # post-DMA dispatch burst with a second tiny-matmul string ----
        warm = psS.tile([P, 1024], F32, tag="S", name="warm")
        for w in range(40):
            nc.tensor.matmul(
                warm[0:1, 0:1],
                lhsT=ones[:, 0:1],
                rhs=ones[:, 0:1],
                start=(w == 0),
                stop=(w == 39),
            )
        clk["pe"] = x_rdy[("k", 0, 0)] + 100
        # second warmup string at the post-DMA-stall dispatch point: the
        # burst after a stall is priced at the LOW p-state; burn it on N=1
        # matmuls so the real projections dispatch at MID/FULL rate
        for w in range(32):
            nc.tensor.matmul(
                warm[0:8, 2:3],
                lhsT=xk[0][0][:, 0, 0:8],
                rhs=xk[0][0][:, 0, 0:1],
                start=(w == 0),
                stop=(w == 31),
            )

        # ---- prologue: k(0,0) on bank4 and q(0,0) on the bank-5 region
        # concurrently (they would otherwise serialize on one bank) ----
        kq = mk_qk("k", xk[0], wk_sb, bk_sb, khB, 0, 0)
        for kc in range(4):
            kq[kc][1]()
        # q(0,0): two 256-token half-quanta through the bank-5 C region
        for th in range(2):
            nc.vector.memset(b5[:, 0:GC], 0.0)
            clk["dve"] = max(clk["dve"], regC_free[0]) + 400
            ms = clk["dve"]
            for kc in range(8):
                tag(nc.tensor.matmul(
                    b5[:, 0:GC],
                    lhsT=wq_sb[:, kc, 0:P],
                    rhs=xq[0][kc // 4][:, kc % 4,
                                       th * 256 : (th + 1) * 256],
                    start=False,
                    stop=(kc == 7),
                    skip_group_check=True,
                ), f"qproj:0.0.{kc}")
                pe(GC * PE_ROW + 3,
                   max(x_rdy[("q", 0, kc // 4)], ms if kc == 0 else 0.0))
                if kc == 3:
                    kq[4 + th * 2][1]()
                    kq[5 + th * 2][1]()
            fin = drain_op(
                420.0, 460.0, clk["pe"] + 150,
                lambda: nc.scalar.add(
                    qhB[0][0][:, th * 256 : (th + 1) * 256],
                    b5[:, 0:GC], bq_sb[:, 0:1]),
                lambda: nc.vector.tensor_scalar_add(
                    qhB[0][0][:, th * 256 : (th + 1) * 256],
                    b5[:, 0:GC], bq_sb[:, 0:1]),
            )
            regC_free[0] = fin
        qh_rdy[(0, 0)] = fin + 100

        # ---- main loop ----
        accs = {}
        acc8 = None
        for p in range(8):
            g = GS[p]
            if p >= 1:
                gprev = GS[p - 1]
                accs[gprev] = psA.tile(
                    [P, 8, HD], F32, tag="acc", name=f"acc{p}"
                )
                # memset this group's rowsum corner (Pool) before first use
                par = (2 * gprev[0] + gprev[1]) % 2
                nc.vector.memset(b5[:, 496 + 8 * par : 504 + 8 * par], 0.0)
                clk["dve"] += 150
            if p == 7:
                acc8 = psA.tile([P, 8, HD], F32, tag="acc", name="acc8")
                par = (2 * g[0] + g[1]) % 2
                nc.vector.memset(b5[:, 496 + 8 * par : 504 + 8 * par], 0.0)
                clk["dve"] += 150
            for i in range(16):
                s = 16 * p + i
                if i >= 1:
                    emit_transpose()
                # hard prerequisites: producers must be EMITTED before use
                need = [
                    lambda: (g[1], i // 4) in kh_rdy,
                    lambda: (g[1], g[0]) in qh_rdy,
                ]
                if p >= 1:
                    need.append(lambda: i in vh_rdy)
                for cond in need:
                    while not cond():
                        # horizon below INF: never pop a quantum whose
                        # producer has not been emitted yet (ready=INF)
                        if not emit_filler(INF / 2):
                            raise RuntimeError("filler starvation")
                if p >= 1:
                    av_step(GS[p - 1], i, accs[GS[p - 1]])
                # rate-controlled fillers before this step's S matmuls
                gate = max(
                    exp_done.get(s - 2, 0.0),
                    kh_rdy[(g[1], i // 4)],
                    qh_rdy[(g[1], g[0])],
                )
                while clk["pe"] + 180 < gate - EPS:
                    if not emit_filler(gate):
                        break
                S, pt, s_done = s_step(g, i)
                emit_exp(g, i, S, pt, s_done, s)
            if p >= 1:
                acc_prev = accs.pop(GS[p - 1])
                drain_norm(GS[p - 1], acc_prev)

        # ---- epilogue: dense AV burst for the last group (its exps are
        # already done or in flight), drain(GS[6]) transposes and leftover
        # fillers interleaved, then drain(GS[7]) with per-chunk pipelined
        # out-proj j=3 through the freed S-ring banks ----
        poE = {}

        def psj3_kk0(cb):
            """Pre-start j3 out-proj with the early-available attT[0]."""
            po = psS.tile([P, 1024], F32, tag="S", name=f"poE{cb}")
            poE[cb] = po
            for e2 in range(2):
                nc.tensor.matmul(
                    po[:, e2 * 512 : (e2 + 1) * 512],
                    lhsT=attT[0][cb][:],
                    rhs=wo_sb[:, 0, e2 * 512 : (e2 + 1) * 512],
                    start=True,
                    stop=False,
                )
            pe(2 * 512 * PE_ROW + 6, attT_rdy[(0, cb)])

        def psj3(cb):
            if cb in poE:
                po = poE[cb]
                for e2 in range(2):
                    nc.tensor.matmul(
                        po[:, e2 * 512 : (e2 + 1) * 512],
                        lhsT=attT[1][cb][:],
                        rhs=wo_sb[:, 1, e2 * 512 : (e2 + 1) * 512],
                        start=False,
                        stop=True,
                    )
                t = pe(2 * 512 * PE_ROW + 6, attT_rdy[(1, cb)])
            else:
                po = psS.tile([P, 1024], F32, tag="S", name=f"poE{cb}")
                for e2 in range(2):
                    for kk in range(2):
                        nc.tensor.matmul(
                            po[:, e2 * 512 : (e2 + 1) * 512],
                            lhsT=attT[kk][cb][:],
                            rhs=wo_sb[:, kk, e2 * 512 : (e2 + 1) * 512],
                            start=(kk == 0),
                            stop=(kk == 1),
                        )
                t = pe(4 * 512 * PE_ROW + 12,
                       max(attT_rdy[(0, cb)], attT_rdy[(1, cb)]))
            ost = opool.tile([P, 1024], BF16, tag="ostE", name=f"ostE{cb}")
            if cb >= 14:
                # last chunk: halve the copy across ACT+DVE and the DMA
                # across SP+Pool to shorten the final drain chain
                nc.scalar.copy(out=ost[:, 0:512], in_=po[:, 0:512])
                nc.vector.tensor_copy(out=ost[:, 512:1024],
                                      in_=po[:, 512:1024])
                clk["act"] = max(clk["act"], t + 150) + 650
                clk["dve"] = max(clk["dve"], t + 150) + 700
                nc.sync.dma_start(out[cb * P : (cb + 1) * P, 0:512],
                                  ost[:, 0:512])
                if cb == 14:
                    nc.gpsimd.dma_start(
                        out[cb * P : (cb + 1) * P, 512:1024],
                        ost[:, 512:1024])
                else:
                    nc.sync.dma_start(
                        out[cb * P : (cb + 1) * P, 512:1024],
                        ost[:, 512:1024])
                return
            fin = drain_op(
                1230.0, 1250.0, t + 150,
                lambda: nc.scalar.copy(out=ost[:], in_=po[:]),
                lambda: nc.vector.tensor_copy(out=ost[:], in_=po[:]),
            )
            if cb == 13:
                nc.gpsimd.dma_start(out[cb * P : (cb + 1) * P, :], ost[:])
                clk["pool"] = max(clk["pool"], fin) + 900
            else:
                nc.sync.dma_start(out[cb * P : (cb + 1) * P, :], ost[:])
                clk["sp"] = max(clk["sp"], fin) + 900

        for i in range(16):
            av_step(GS[7], i, acc8)
            emit_transpose()
            emit_filler(INF / 2)
        drain_norm(GS[7], acc8)
        # psS has 2 buffers: pre-start the first two chunks on attT[0]
        psj3_kk0(12)
        psj3_kk0(13)
        while fillers or opens["4"] is not None or opens["C"] is not None:
            if not emit_filler(INF / 2):
                break
        for ch in range(4):
            emit_transpose()
            psj3(12 + ch)
        while emit_transpose():
            pass


_NC_CACHE: list = []


def kernel(q, k, v, Wq, bq, Wk, bk, Wv, bv, Wo, bo):
    q = np.asarray(q, dtype=np.float32)
    k = np.asarray(k, dtype=np.float32)
    v = np.asarray(v, dtype=np.float32)
    Wq = np.asarray(Wq, dtype=np.float32)
    Wk = np.asarray(Wk, dtype=np.float32)
    Wv = np.asarray(Wv, dtype=np.float32)
    Wo = np.asarray(Wo, dtype=np.float32)
    bq = np.asarray(bq, dtype=np.float32)
    bk = np.asarray(bk, dtype=np.float32)
    bv = np.asarray(bv, dtype=np.float32)
    bo = np.asarray(bo, dtype=np.float32)

    if not _NC_CACHE:
        _NC_CACHE.append(build_nc())
    nc = _NC_CACHE[0]

    bf = ml_dtypes.bfloat16
    qTb = [np.ascontiguousarray(q[b].T).astype(bf) for b in range(B)]
    kTb = [np.ascontiguousarray(k[b].T).astype(bf) for b in range(B)]
    vTb = [np.ascontiguousarray(v[b].T).astype(bf) for b in range(B)]

    in_maps = []
    for c in range(NC_):
        b, g = divmod(c, GROUPS)
        cs = slice(g * GC, (g + 1) * GC)
        in_maps.append(
            {
                "qT": qTb[b],
                "kT": kTb[b],
                "vT": vTb[b],
                "wq": Wq[:, cs].astype(bf),
                "wk": Wk[:, cs].astype(bf),
                "wv": Wv[:, cs].astype(bf),
                "wo": np.ascontiguousarray(Wo[cs, :]).astype(bf),
                "bq": bq[cs],
                "bk": bk[cs],
                "ident": np.eye(P, dtype=bf),
            }
        )

    kw = {}
    if TRACE:
        kw = dict(trace=True, tmpdir=TRACE_DIR, **TRACE_KW)
    res = run_bass_kernel_spmd(nc, in_maps, core_ids=list(range(NC_)), **kw)
    LAST_RESULT.clear()
    LAST_RESULT.append(res)

    outp = np.zeros((B, T, E), dtype=np.float32)
    for c in range(NC_):
        b = c // GROUPS
        outp[b] += res.results[c]["out"].astype(np.float32)
    # bv's contribution (softmax rows sum to 1): (1 . bv^T) @ Wo, plus bo
    outp += bo + bv @ Wo
    return outp


TRACE = False
TRACE_DIR = None
TRACE_KW: dict = {}
LAST_RESULT: list = []
EMIT_LOG: dict = {}
